# revision 1
# baseline (speedup 1.0000x reference)
"""FFTBias Trainium2 kernel (self-contained).

Math: the reference computes, per head h and lane (b,d):
    z = exp(w_circ - offset);  Z = rfft_4091(z)
    pbv = irfft_4090(rfft_4091(leftpad(v)) * Z)[:, :S]
    z_pb = irfft_4090(Z * rfft_4091(leftpad(ones))) [:, :S]
with S=2046, N1=2*S-1=4091 (prime), N2=2*S-2=4090 (mismatched irfft length).

Key identity: z is symmetric about index S (mod N1), so Z[k] = w^{kS} R[k]
with R real, and the left-pad phase w^{k*pad} satisfies pad + S = N1, i.e.
the phases cancel exactly:
    P[k] = X[k]Z[k] = R[k] * sum_s v[s] w^{ks}
    R[k] = 2 * sum_j Fc[k,j] * eh[j],  eh = exp(w - offset) with eh[0] halved
so the whole op is 4 dense real matmuls with DFT tables:
    y = Ic @ (R .* (Fc @ v)) + Is @ (R .* (Fs @ v))
where Fc/Fs are cos/sin(2*pi*k*s/4091) and Ic/Is are the c_k-weighted
cos/sin(2*pi*t*k/4090) irfft tables (pocketfft c2r semantics: DC/Nyquist
imag parts drop out as sin()=0 there). z_pb rides along as 12 extra
columns via the constant Dirichlet vector D[k] = sum_s w^{ks}.

Sharding: data-parallel over batch, one batch per core; every core
redundantly computes the tiny z_pb path (host takes core 0's).
"""

import numpy as np

import concourse.bass as bass
import concourse.mybir as mybir
import concourse.tile as tile
from concourse.bass_utils import run_bass_kernel_spmd

F32 = mybir.dt.float32
F32R = mybir.dt.float32r
F16 = mybir.dt.float16
BF16 = mybir.dt.bfloat16

S = 2046
N1 = 4091
N2 = 4090
H = 12
D = 64
B = 8
SP = 2048          # padded sequence (16 chunks of 128)
KCH = 16
LN = H * D         # 768 main lanes
LW = LN + H        # 780 = main + z_pb columns
ALPHA = 2.0 ** -13  # scale folded into g to keep fp16 in range
NCORES = 8

MM_DTYPE = "f32r"   # "f32r" | "f16" | "bf16" | "f32"

_MMDT = {"f32r": F32R, "f16": F16, "bf16": BF16, "f32": F32}
_NPDT = {"f32r": np.float32, "f16": np.float16, "bf16": np.float32, "f32": np.float32}

_tables_cache = {}
_nc_cache = {}


def _build_tables(mm):
    """Host-side constant DFT tables, tiled for [128s|k, 128bin|t] lhsT use."""
    if mm in _tables_cache:
        return _tables_cache[mm]
    k = np.arange(S, dtype=np.float64)
    # forward tables: cos/sin(2*pi*k*s/N1), exact integer mod for angle precision
    ks = np.mod(np.outer(k, k), N1)
    ang1 = (2.0 * np.pi / N1) * ks
    Fc = np.cos(ang1)
    Fs = np.sin(ang1)
    # Dirichlet vector D[k] = sum_s w^{ks} (before padding)
    Dr = Fc.sum(axis=1)
    Ds = Fs.sum(axis=1)
    # inverse tables with c_k weights, x2 (R fold) and 1/ALPHA (g scale fold)
    tk = np.mod(np.outer(k, k), N2)
    ang2 = (2.0 * np.pi / N2) * tk
    ck = np.full(S, 2.0 / N2)
    ck[0] = 1.0 / N2
    ck[S - 1] = 1.0 / N2
    scale = 2.0 / ALPHA
    Ic = scale * ck[None, :] * np.cos(ang2)
    Is = scale * ck[None, :] * np.sin(ang2)

    npdt = _NPDT[mm]

    def pad(a):
        out = np.zeros((SP, SP), np.float64)
        out[:S, :S] = a
        return out

    def fwd_tiled(a):
        # T[m, p, kc*128+c] = a_pad[kc*128+p, m*128+c]
        a4 = pad(a).reshape(KCH, 128, KCH, 128)
        return np.ascontiguousarray(
            a4.transpose(2, 1, 0, 3).reshape(KCH * 128, KCH * 128)
        ).astype(npdt)

    def inv_tiled(a):
        # T[m, p, kc*128+c] = a_pad[m*128+c, kc*128+p]
        a4 = pad(a).reshape(KCH, 128, KCH, 128)
        return np.ascontiguousarray(
            a4.transpose(0, 3, 2, 1).reshape(KCH * 128, KCH * 128)
        ).astype(npdt)

    TFc = fwd_tiled(Fc)
    TFs = fwd_tiled(Fs)
    TIc = inv_tiled(Ic)
    TIs = inv_tiled(Is)
    Dv = np.zeros((SP, 2), np.float32)
    Dv[:S, 0] = Dr
    Dv[:S, 1] = Ds
    _tables_cache[mm] = (TFc, TFs, TIc, TIs, Dv)
    return _tables_cache[mm]


def _split_excess_waits(nc):
    """This walrus caps sync waits at 1/instruction (2 for EventSemaphore);
    hoist extras onto standalone NoOps on the same engine just before."""
    n = 0
    for f in nc.m.functions:
        for bb in f.blocks:
            new_insts = []
            for inst in bb.instructions:
                si = getattr(inst, "sync_info", None)
                ow = list(si.on_wait) if si and si.on_wait else []
                cap = 2 if isinstance(inst, mybir.InstEventSemaphore) else 1
                if len(ow) > cap:
                    extra, keep = ow[:-cap], ow[-cap:]
                    for w in extra:
                        nop = mybir.InstNoOp(
                            name=f"I-waitfix-{n}",
                            engine=inst.engine,
                            sync_info=mybir.SyncInfo(on_wait=[w], on_update=[]),
                        )
                        n += 1
                        nc.register_instruction(nop)
                        new_insts.append(nop)
                    si.on_wait = keep
                new_insts.append(inst)
            bb.instructions[:] = new_insts
    return n


def _inline_mm_tensor(nc, data, name, mmdt):
    """inline const usable as a matmul operand of dtype mmdt."""
    h = nc.inline_tensor(np.ascontiguousarray(data), name)
    if mmdt == F32R:
        nc.lookup_mls(h).dtype = F32R
        h = bass.DRamTensorHandle(name, list(data.shape), F32R)
    return h


def build_nc(mm=MM_DTYPE):
    if mm in _nc_cache:
        return _nc_cache[mm]
    mmdt = _MMDT[mm]
    TFc, TFs, TIc, TIs, Dv = _build_tables(mm)

    nc = bass.Bass()
    vb_d = nc.dram_tensor("vb", [SP, LN], F32, kind="ExternalInput")
    wt_d = nc.dram_tensor("wt", [SP, H], F32, kind="ExternalInput")
    offx_d = nc.dram_tensor("offx", [2, 128, H], F32, kind="ExternalInput")
    y_d = nc.dram_tensor("y", [SP, LN], F32, kind="ExternalOutput")
    yz_d = nc.dram_tensor("yz", [SP, H], F32, kind="ExternalOutput")

    tfc_d = _inline_mm_tensor(nc, TFc, "TFc", mmdt)
    tfs_d = _inline_mm_tensor(nc, TFs, "TFs", mmdt)
    tic_d = _inline_mm_tensor(nc, TIc, "TIc", mmdt)
    tis_d = _inline_mm_tensor(nc, TIs, "TIs", mmdt)
    dv_d = nc.inline_tensor(Dv, "Dv")

    MULT = mybir.AluOpType.mult
    EXP = mybir.ActivationFunctionType.Exp

    with tile.TileContext(nc) as tc:
        with (
            tc.tile_pool(name="cpool", bufs=1) as cpool,
            tc.tile_pool(name="data", bufs=1) as data,
            tc.tile_pool(name="stg", bufs=2) as stg,
            tc.tile_pool(name="wstg", bufs=2) as wstg,
            tc.tile_pool(name="tbl", bufs=4) as tbl,
            tc.tile_pool(name="ypool", bufs=2) as ypool,
            tc.tile_pool(name="ps", bufs=4, space="PSUM") as pspool,
        ):
            # constants
            dv_sb = cpool.tile([128, KCH, 2], F32)
            nc.sync.dma_start(dv_sb[:], dv_d[:].rearrange("(c p) x -> p c x", p=128))
            offx_sb = cpool.tile([128, 2, H], F32)
            nc.sync.dma_start(offx_sb[:, 0, :], offx_d[0, :, :])
            nc.sync.dma_start(offx_sb[:, 1, :], offx_d[1, :, :])
            zb = cpool.tile([128, 1], F32)
            nc.gpsimd.memset(zb[:], 0.0)

            # ---- load v (cast) and compute eh = exp(wt - off) ----
            v_sb = data.tile([128, KCH, LW], mmdt)
            for kc in range(KCH):
                st = stg.tile([128, LN], F32)
                nc.sync.dma_start(st[:], vb_d[kc * 128:(kc + 1) * 128, :])
                nc.vector.tensor_copy(v_sb[:, kc, 0:LN], st[:])
                wtt = wstg.tile([128, H], F32)
                nc.sync.dma_start(wtt[:], wt_d[kc * 128:(kc + 1) * 128, :])
                sb2 = wstg.tile([128, H], F32)
                nc.vector.tensor_sub(
                    sb2[:], wtt[:], offx_sb[:, 0 if kc == 0 else 1, :]
                )
                ex = wstg.tile([128, H], F32)
                nc.scalar.activation(ex[:], sb2[:], EXP, bias=zb[:])
                nc.vector.tensor_copy(v_sb[:, kc, LN:LW], ex[:])

            g_c = data.tile([128, KCH, LW], mmdt)
            g_s = data.tile([128, KCH, LW], mmdt)

            # ---- phase A: forward DFT + R-scale ----
            for m in range(KCH):
                fct = tbl.tile([128, KCH * 128], mmdt, tag="tbl")
                nc.sync.dma_start(fct[:], tfc_d[m * 128:(m + 1) * 128, :])
                fst = tbl.tile([128, KCH * 128], mmdt, tag="tbl")
                nc.sync.dma_start(fst[:], tfs_d[m * 128:(m + 1) * 128, :])
                psc = pspool.tile([128, LW], F32, tag="ps")
                pss = pspool.tile([128, LW], F32, tag="ps")
                for kc in range(KCH):
                    lc = fct[:, kc * 128:(kc + 1) * 128]
                    ls = fst[:, kc * 128:(kc + 1) * 128]
                    r0 = v_sb[:, kc, 0:512]
                    r1 = v_sb[:, kc, 512:LW]
                    st0, sp0 = kc == 0, kc == KCH - 1
                    nc.tensor.matmul(psc[:, 0:512], lc, r0, start=st0, stop=sp0)
                    nc.tensor.matmul(psc[:, 512:LW], lc, r1, start=st0, stop=sp0)
                    nc.tensor.matmul(pss[:, 0:512], ls, r0, start=st0, stop=sp0)
                    nc.tensor.matmul(pss[:, 512:LW], ls, r1, start=st0, stop=sp0)
                # R (= R/2 actually; x2 folded into Ic/Is) to SBUF
                rsb = wstg.tile([128, H], F32, tag="rsb")
                nc.vector.tensor_copy(rsb[:], psc[:, LN:LW])
                rb = rsb[:].unsqueeze(2).to_broadcast((128, H, D))
                nc.vector.scalar_tensor_tensor(
                    g_c[:, m, 0:LN].rearrange("p (h d) -> p h d", d=D),
                    psc[:, 0:LN].rearrange("p (h d) -> p h d", d=D),
                    ALPHA, rb, MULT, MULT,
                )
                nc.vector.scalar_tensor_tensor(
                    g_s[:, m, 0:LN].rearrange("p (h d) -> p h d", d=D),
                    pss[:, 0:LN].rearrange("p (h d) -> p h d", d=D),
                    ALPHA, rb, MULT, MULT,
                )
                nc.vector.tensor_scalar(
                    g_c[:, m, LN:LW], rsb[:], dv_sb[:, m, 0:1], ALPHA, MULT, MULT
                )
                nc.vector.tensor_scalar(
                    g_s[:, m, LN:LW], rsb[:], dv_sb[:, m, 1:2], ALPHA, MULT, MULT
                )

            # ---- phase B: inverse DFT ----
            for m in range(KCH):
                ict = tbl.tile([128, KCH * 128], mmdt, tag="tbl")
                nc.sync.dma_start(ict[:], tic_d[m * 128:(m + 1) * 128, :])
                ist = tbl.tile([128, KCH * 128], mmdt, tag="tbl")
                nc.sync.dma_start(ist[:], tis_d[m * 128:(m + 1) * 128, :])
                psy = pspool.tile([128, LW], F32, tag="ps")
                for kc in range(KCH):
                    lc = ict[:, kc * 128:(kc + 1) * 128]
                    ls = ist[:, kc * 128:(kc + 1) * 128]
                    st0 = kc == 0
                    sp0 = kc == KCH - 1
                    nc.tensor.matmul(psy[:, 0:512], lc, g_c[:, kc, 0:512],
                                     start=st0, stop=False)
                    nc.tensor.matmul(psy[:, 512:LW], lc, g_c[:, kc, 512:LW],
                                     start=st0, stop=False)
                    nc.tensor.matmul(psy[:, 0:512], ls, g_s[:, kc, 0:512],
                                     start=False, stop=sp0)
                    nc.tensor.matmul(psy[:, 512:LW], ls, g_s[:, kc, 512:LW],
                                     start=False, stop=sp0)
                y = ypool.tile([128, LW], F32)
                nc.scalar.copy(y[:], psy[:])
                nc.sync.dma_start(y_d[m * 128:(m + 1) * 128, :], y[:, 0:LN])
                nc.sync.dma_start(yz_d[m * 128:(m + 1) * 128, :], y[:, LN:LW])

    _split_excess_waits(nc)
    _nc_cache[mm] = nc
    return nc


def _prep_inputs(v, offset, w):
    v = np.asarray(v, np.float32)
    offset = np.asarray(offset, np.float32)
    w = np.asarray(w, np.float32)
    wt = np.zeros((SP, H), np.float32)
    wt[:S, :] = w[0].T
    offx = np.zeros((2, 128, H), np.float32)
    offx[0] = offset[0][None, :]
    offx[0, 0, :] += np.float32(np.log(2.0))  # halve eh[0] (j = 0 lives in chunk 0 row 0)
    offx[1] = offset[0][None, :]
    in_maps = []
    for b in range(B):
        vb = np.zeros((SP, LN), np.float32)
        vb[:S, :] = v[b, 1:S + 1].reshape(S, LN)
        in_maps.append({"vb": vb, "wt": wt, "offx": offx})
    return in_maps


def kernel(v, offset, w, _mm=None, _trace=False):
    mm = _mm or MM_DTYPE
    nc = build_nc(mm)
    in_maps = _prep_inputs(v, offset, w)
    res = run_bass_kernel_spmd(nc, in_maps, core_ids=list(range(NCORES)),
                               trace=_trace)
    pbv = np.zeros((B, SP, H, D), np.float32)
    for b in range(B):
        pbv[b, 1:S + 1] = res.results[b]["y"][:S].reshape(S, H, D)
    z_pb = np.zeros((1, SP, H), np.float32)
    z_pb[0, 1:S + 1] = res.results[0]["yz"][:S]
    return pbv, z_pb
